# revision 51
# baseline (speedup 1.0000x reference)
"""Trainium2 Bass kernel for nn_ModAttn (modulated multi-function attention).

Shapes: x [1,1024,512], compatibility [1,4,1024]; out [1,4,1024,512].

v2c design:
- Sharding: 8 cores = (function f in 0..3) x (head-half hh in 0..1). Each core
  computes 4 heads over all 1024 tokens and a PARTIAL output projection
  (contraction over its 256 y_hat dims). Host sums the two partials per
  function and adds the (host-folded) bias.
- Host folds the modulation: cm = layernorm(w_c@code) computed in numpy and
  multiplied into W_qkv / W_proj per function. v-bias + b_proj fold into one
  host-side bias vector.
- All PE operands bf16 (PSUM accum fp32). Every matmul out <= 512 fp32 cols.
- Attention in transposed orientation; first softmax denominator via
  ones-matmul; second via ones-column in v.
- Software pipeline at m-tile granularity: D1(h) interleaved with D2(h-1);
  B(t1,t3)/C/v matmuls act as PE filler during head 0; one rotating PSUM
  pool shared by all big matmul outputs.
"""

import os
import numpy as np
from contextlib import ExitStack

DEBUG_DUMP = bool(int(os.environ.get("MODATTN_DEBUG", "0")))
N_CORES = 8
N, DIN, NF, H, HD = 1024, 512, 4, 8, 64
HPC = 4            # heads per core
HDIM = HPC * HD    # 256 qkv dims per core
SCALE = HD ** -0.5

_CACHE = {}


def build_nc():
    import concourse.bacc as bacc
    import concourse.tile as tile
    from concourse import mybir

    F32 = mybir.dt.float32
    BF16 = mybir.dt.bfloat16
    AT = mybir.ActivationFunctionType

    nc = bacc.Bacc("TRN2", target_bir_lowering=False, debug=False,
                   num_devices=N_CORES)

    xt_d = nc.dram_tensor("xt", [DIN, N], BF16, kind="ExternalInput")
    wq_d = nc.dram_tensor("wq", [DIN, 3 * HDIM], BF16, kind="ExternalInput")
    wp_d = nc.dram_tensor("wp", [HDIM, DIN], BF16, kind="ExternalInput")
    cmat_d = nc.dram_tensor("cmat", [N, N], BF16, kind="ExternalInput")
    bqk_d = nc.dram_tensor("bqk", [2 * HDIM], F32, kind="ExternalInput")
    y_d = nc.dram_tensor("y", [N, DIN], BF16, kind="ExternalOutput")

    with tile.TileContext(nc) as tc, ExitStack() as top:
        const = top.enter_context(tc.tile_pool(name="const", bufs=1))
        ones_bf = const.tile([128, 1], BF16, tag="ones_bf")
        nc.vector.memset(ones_bf[:], 1.0)
        scr = const.tile([128, 512], BF16, tag="scr")
        nc.vector.memset(scr[:], 0.0)
        bqk_t = const.tile([128, 4], F32, tag="bqk")

        big = top.enter_context(tc.tile_pool(name="big", bufs=1))
        wq = [big.tile([128, 3 * HDIM], BF16, tag=f"wq{c}", name=f"wq{c}")
              for c in range(4)]
        xt = [big.tile([128, N], BF16, tag=f"xt{c}", name=f"xt{c}")
              for c in range(4)]
        wp = [big.tile([128, DIN], BF16, tag=f"wp{c}", name=f"wp{c}")
              for c in range(2)]
        # spread big loads across DMA queues so they land in parallel
        qeng = [nc.gpsimd, nc.sync, nc.scalar]
        for c in range(4):
            qeng[c % 3].dma_start(wq[c][:], wq_d.ap()[c * 128:(c + 1) * 128, :])
            qeng[(c + 1) % 3].dma_start(xt[c][:],
                                        xt_d.ap()[c * 128:(c + 1) * 128, :])
        for j in range(4):
            nc.sync.dma_start(bqk_t[:, j:j + 1],
                              bqk_d.ap()[j * 128:(j + 1) * 128])
        for c in range(2):
            qeng[c].dma_start(wp[c][:], wp_d.ap()[c * 128:(c + 1) * 128, :])

        per = top.enter_context(tc.tile_pool(name="per", bufs=1))
        qkT = [per.tile([128, N], BF16, tag=f"qkT{t}", name=f"qkT{t}")
               for t in range(4)]
        VW = 128  # per-head stationary width: [ones | 63 pad | 64 v dims]
        vv = [per.tile([128, HPC * VW], BF16, tag=f"vv{m}", name=f"vv{m}")
              for m in range(8)]
        Ct = [per.tile([128, N], BF16, tag=f"C{m}", name=f"C{m}")
              for m in range(8)]
        ymT = [per.tile([128, N], BF16, tag=f"ymT{t}", name=f"ymT{t}")
               for t in range(2)]
        for m in range(8):
            qeng[m % 3].dma_start(Ct[m][:], cmat_d.ap()[m * 128:(m + 1) * 128, :])

        with tc.tile_pool(name="psMM", bufs=2, space="PSUM") as psMM, \
             tc.tile_pool(name="psZ", bufs=2, space="PSUM") as psZ, \
             tc.tile_pool(name="psY", bufs=2, space="PSUM") as psY, \
             tc.tile_pool(name="smE1", bufs=16) as smE1, \
             tc.tile_pool(name="smT1", bufs=4) as smT1, \
             tc.tile_pool(name="smT2", bufs=3) as smT2, \
             tc.tile_pool(name="smE2", bufs=4) as smE2, \
             tc.tile_pool(name="smZ", bufs=2) as smZ, \
             tc.tile_pool(name="smO", bufs=4) as smO:

            # HAM pre-warm: ~6us of back-to-back dummy matmuls that only
            # depend on memset data, overlapping the input DMA wait.
            wps = psMM.tile([128, N], F32, tag="ps", name="ps")
            for _ in range(20):
                nc.tensor.matmul(wps[0:1, 0:512], ones_bf[:], scr[:],
                                 start=True, stop=True)

            # ---------- emission helpers ----------
            def emit_qk_tile(t):
                ps = psMM.tile([128, N], F32, tag="ps", name="ps")
                for half in range(2):
                    for c in range(4):
                        nc.tensor.matmul(
                            ps[:, half * 512:(half + 1) * 512],
                            wq[c][:, t * 128:(t + 1) * 128],
                            xt[c][:, half * 512:(half + 1) * 512],
                            start=(c == 0), stop=(c == 3))
                nc.scalar.activation(qkT[t][:], ps[:], AT.Identity,
                                     bias=bqk_t[:, t:t + 1])

            def emit_v_pair(g):  # 2 token-tiles of v per psMM slot
                ps = psMM.tile([128, N], F32, tag="ps", name="ps")
                for i in range(2):
                    m = 2 * g + i
                    pv = ps[:, i * HDIM:(i + 1) * HDIM]
                    for c in range(4):
                        nc.tensor.matmul(pv, xt[c][:, m * 128:(m + 1) * 128],
                                         wq[c][:, 2 * HDIM:3 * HDIM],
                                         start=(c == 0), stop=(c == 3))
                for i in range(2):
                    m = 2 * g + i
                    v3 = vv[m][:].rearrange("p (h e) -> p h e", e=VW)
                    nc.vector.tensor_copy(
                        v3[:, :, 64:VW],
                        ps[:, i * HDIM:(i + 1) * HDIM].rearrange(
                            "p (h e) -> p h e", e=HD))
                    nc.gpsimd.memset(v3[:, :, 0:1], 1.0)
                    nc.gpsimd.memset(v3[:, :, 1:64], 0.0)

            state = {}

            def emit_scores(h, m):
                hp, ho = h // 2, (h % 2) * 64
                st = state[h]
                ps = psMM.tile([128, N], F32, tag="ps", name="ps")
                e1 = smE1.tile([128, N], BF16, tag="e1", name="e1")
                for qh in range(2):
                    nc.tensor.matmul(
                        ps[:, qh * 512:(qh + 1) * 512],
                        qkT[2 + hp][ho:ho + 64, m * 128:(m + 1) * 128],
                        qkT[hp][ho:ho + 64, qh * 512:(qh + 1) * 512],
                        start=True, stop=True)
                nc.scalar.activation(e1[:], ps[:], AT.Exp, scale=SCALE)
                st["e1"][m] = e1

            def emit_sum(h, m):
                st = state[h]
                e1 = st["e1"][m]
                for qh in range(2):
                    nc.tensor.matmul(st["s_ps"][qh][:], ones_bf[:],
                                     e1[:, qh * 512:(qh + 1) * 512],
                                     start=(m == 0), stop=(m == 7))

            def emit_rsb(h):
                st = state[h]
                rs = smZ.tile([1, N], F32, tag="rs", name="rs")
                for qh in range(2):
                    nc.vector.reciprocal_approx_fast(
                        rs[:, qh * 512:(qh + 1) * 512], st["s_ps"][qh][:])
                rs16 = smZ.tile([1, N], BF16, tag="rs16", name="rs16")
                nc.vector.tensor_copy(rs16[:], rs[:])
                rsb = smZ.tile([128, N], BF16, tag="rsb", name="rsb")
                nc.gpsimd.partition_broadcast(rsb[:], rs16[:], channels=128)
                st["rsb"] = rsb
                st["ypv"] = [psY.tile([VW, 512], F32, tag="ypv",
                                      name="ypv") for _ in range(2)]

            def emit_d2_step(h, m):
                st = state[h]
                t1 = smT1.tile([128, N], BF16, tag="t1", name="t1")
                nc.vector.tensor_mul(t1[:], st["e1"][m][:], st["rsb"][:])
                if m % 2 == 0:
                    st["t2p"] = smT2.tile([128, 2 * N], BF16, tag="t2p",
                                          name="t2p")
                sub = m % 2
                t2s = st["t2p"][:, sub * N:(sub + 1) * N]
                nc.vector.tensor_mul(t2s, t1[:], Ct[m][:])
                if sub == 1:
                    e2 = smE2.tile([128, 2 * N], BF16, tag="e2", name="e2")
                    nc.scalar.activation(e2[:], st["t2p"][:], AT.Exp)
                    for s2 in range(2):
                        mm = m - 1 + s2
                        for qh in range(2):
                            nc.tensor.matmul(
                                st["ypv"][qh][:],
                                vv[mm][:, h * VW:(h + 1) * VW],
                                e2[:, s2 * N + qh * 512:s2 * N + (qh + 1) * 512],
                                start=(mm == 0), stop=(mm == 7))

            def emit_ztail(h):
                hp, ho = h // 2, (h % 2) * 64
                st = state.pop(h)
                ypv = st["ypv"]
                rz = smZ.tile([1, N], F32, tag="rz", name="rz")
                for qh in range(2):
                    nc.vector.reciprocal_approx_fast(
                        rz[:, qh * 512:(qh + 1) * 512], ypv[qh][0:1, :])
                zb = smZ.tile([64, N], F32, tag="zb", name="zb")
                nc.gpsimd.partition_broadcast(zb[:], rz[:], channels=64)
                for qh in range(2):
                    nc.vector.tensor_mul(
                        ymT[hp][ho:ho + 64, qh * 512:(qh + 1) * 512],
                        ypv[qh][64:VW, :], zb[:, qh * 512:(qh + 1) * 512])

            # ---------- schedule ----------
            emit_qk_tile(0)
            emit_qk_tile(2)

            filler = ([lambda t=1: emit_qk_tile(t),
                       lambda t=3: emit_qk_tile(t)]
                      + [lambda g=g: emit_v_pair(g) for g in range(4)])

            for h in range(HPC):
                state[h] = {"e1": {}, "s_ps": [
                    psZ.tile([1, 512], F32, tag="s_ps", name="s_ps")
                    for _ in range(2)]}
                for m in range(8):
                    emit_scores(h, m)
                    if m >= 1:
                        emit_sum(h, m - 1)
                    if filler:
                        filler.pop(0)()
                    if h >= 1 and m >= 2:
                        emit_d2_step(h - 1, m - 2)
                emit_sum(h, 7)
                if h >= 1:
                    emit_d2_step(h - 1, 6)
                    emit_d2_step(h - 1, 7)
                    emit_ztail(h - 1)
                emit_rsb(h)
            # ---------- E: partial output projection (pipelined with D2 h3)
            e_ps = {}

            def emit_e_chunk0(np_):
                ps = psMM.tile([128, N], F32, tag="ps", name="ps")
                for i in range(2):
                    nb = 2 * np_ + i
                    nc.tensor.matmul(ps[:, i * 512:(i + 1) * 512],
                                     ymT[0][:, nb * 128:(nb + 1) * 128],
                                     wp[0][:], start=True, stop=False)
                e_ps[np_] = ps

            def emit_e_finish(np_):
                if np_ not in e_ps:
                    emit_e_chunk0(np_)
                ps = e_ps[np_]
                for i in range(2):
                    nb = 2 * np_ + i
                    nc.tensor.matmul(ps[:, i * 512:(i + 1) * 512],
                                     ymT[1][:, nb * 128:(nb + 1) * 128],
                                     wp[1][:], start=False, stop=True)
                yo = smO.tile([128, N], BF16, tag="yo", name="yo")
                if np_ % 2 == 0:
                    nc.scalar.copy(yo[:], ps[:])
                else:
                    nc.vector.tensor_copy(yo[:], ps[:])
                for i in range(2):
                    nb = 2 * np_ + i
                    qeng[nb % 3].dma_start(y_d.ap()[nb * 128:(nb + 1) * 128, :],
                                           yo[:, i * 512:(i + 1) * 512])

            emit_e_chunk0(0)
            emit_e_chunk0(1)
            for m in range(8):
                emit_d2_step(HPC - 1, m)
            emit_ztail(HPC - 1)
            for np_ in range(4):
                emit_e_finish(np_)

    nc.compile()
    return nc


def _host_fold(code, w_c, W_qkv, b_qkv, W_proj, b_proj,
               ln_qkv_g, ln_qkv_b, ln_proj_g, ln_proj_b):
    """Compute modulation vectors on host and fold into weights/biases."""
    cm0 = (w_c @ code).T  # [NF, DIN]
    def ln(v, g, b, eps=1e-5):
        mu = v.mean(-1, keepdims=True)
        var = v.var(-1, keepdims=True)
        return (v - mu) / np.sqrt(var + eps) * g + b
    cmq = ln(cm0, ln_qkv_g, ln_qkv_b)    # [NF, DIN]
    cmp_ = ln(cm0, ln_proj_g, ln_proj_b)
    bv = b_qkv[2 * DIN:3 * DIN]
    bias_f = b_proj[None, :] + (bv[None, :] * cmp_) @ W_proj.T  # [NF, DIN]
    return cmq, cmp_, bias_f


def make_in_maps(x, compatibility, code, w_c, W_qkv, b_qkv, W_proj, b_proj,
                 ln_qkv_g, ln_qkv_b, ln_proj_g, ln_proj_b):
    import ml_dtypes
    bf = ml_dtypes.bfloat16
    x = np.asarray(x, np.float32)
    compatibility = np.asarray(compatibility, np.float32)
    code = np.asarray(code, np.float32)
    w_c = np.asarray(w_c, np.float32)
    W_qkv = np.asarray(W_qkv, np.float32)
    b_qkv = np.asarray(b_qkv, np.float32)
    W_proj = np.asarray(W_proj, np.float32)
    b_proj = np.asarray(b_proj, np.float32)

    cmq, cmp_, bias_f = _host_fold(
        code, w_c, W_qkv, b_qkv, W_proj, b_proj,
        np.asarray(ln_qkv_g, np.float32), np.asarray(ln_qkv_b, np.float32),
        np.asarray(ln_proj_g, np.float32), np.asarray(ln_proj_b, np.float32))

    xT = np.ascontiguousarray(x[0].T.astype(bf))      # [512, 1024] bf16
    compb = compatibility[0].astype(bf).astype(np.float32)
    cmat = np.ascontiguousarray((compb.T @ compb).astype(bf))  # [1024,1024]
    in_maps = []
    for core in range(N_CORES):
        f, hh = core // 2, core % 2
        Wq_f = W_qkv * cmq[f][None, :]                # [1536, 512]
        rows = np.r_[hh * HDIM:(hh + 1) * HDIM,
                     DIN + hh * HDIM:DIN + (hh + 1) * HDIM,
                     2 * DIN + hh * HDIM:2 * DIN + (hh + 1) * HDIM]
        wq = np.ascontiguousarray(Wq_f[rows].T.astype(bf))   # [512, 768]
        Wp_f = W_proj * cmp_[f][None, :]              # [512, 512]
        wp = np.ascontiguousarray(
            Wp_f[:, hh * HDIM:(hh + 1) * HDIM].T.astype(bf))  # [256, 512]
        bqk = np.ascontiguousarray(
            b_qkv[np.r_[hh * HDIM:(hh + 1) * HDIM,
                        DIN + hh * HDIM:DIN + (hh + 1) * HDIM]])  # [512]
        in_maps.append(dict(xt=xT, wq=wq, wp=wp, cmat=cmat, bqk=bqk))
    _CACHE["bias_f"] = bias_f
    return in_maps


def kernel(**inputs) -> np.ndarray:
    from concourse.bass_utils import run_bass_kernel_spmd
    if "nc" not in _CACHE:
        _CACHE["nc"] = build_nc()
    nc = _CACHE["nc"]
    in_maps = make_in_maps(**inputs)
    bias_f = _CACHE["bias_f"]
    res = run_bass_kernel_spmd(nc, in_maps, core_ids=list(range(N_CORES)))
    out = np.empty((1, NF, N, DIN), np.float32)
    for f in range(NF):
        p0 = np.asarray(res.results[2 * f]["y"]).astype(np.float32)
        p1 = np.asarray(res.results[2 * f + 1]["y"]).astype(np.float32)
        out[0, f] = p0 + p1 + bias_f[f][None, :]
    return out


# revision 52
# speedup vs baseline: 1.0140x; 1.0140x over previous
"""Trainium2 Bass kernel for nn_ModAttn (modulated multi-function attention).

Shapes: x [1,1024,512], compatibility [1,4,1024]; out [1,4,1024,512].

v2c design:
- Sharding: 8 cores = (function f in 0..3) x (head-half hh in 0..1). Each core
  computes 4 heads over all 1024 tokens and a PARTIAL output projection
  (contraction over its 256 y_hat dims). Host sums the two partials per
  function and adds the (host-folded) bias.
- Host folds the modulation: cm = layernorm(w_c@code) computed in numpy and
  multiplied into W_qkv / W_proj per function. v-bias + b_proj fold into one
  host-side bias vector.
- All PE operands bf16 (PSUM accum fp32). Every matmul out <= 512 fp32 cols.
- Attention in transposed orientation; first softmax denominator via
  ones-matmul; second via ones-column in v.
- Software pipeline at m-tile granularity: D1(h) interleaved with D2(h-1);
  B(t1,t3)/C/v matmuls act as PE filler during head 0; one rotating PSUM
  pool shared by all big matmul outputs.
"""

import os
import numpy as np
from contextlib import ExitStack

DEBUG_DUMP = bool(int(os.environ.get("MODATTN_DEBUG", "0")))
N_CORES = 8
N, DIN, NF, H, HD = 1024, 512, 4, 8, 64
HPC = 4            # heads per core
HDIM = HPC * HD    # 256 qkv dims per core
SCALE = HD ** -0.5

_CACHE = {}


def build_nc():
    import concourse.bacc as bacc
    import concourse.tile as tile
    from concourse import mybir

    F32 = mybir.dt.float32
    BF16 = mybir.dt.bfloat16
    AT = mybir.ActivationFunctionType

    nc = bacc.Bacc("TRN2", target_bir_lowering=False, debug=False,
                   num_devices=N_CORES)

    xt_d = nc.dram_tensor("xt", [DIN, N], BF16, kind="ExternalInput")
    wq_d = nc.dram_tensor("wq", [DIN, 3 * HDIM], BF16, kind="ExternalInput")
    wp_d = nc.dram_tensor("wp", [HDIM, DIN], BF16, kind="ExternalInput")
    cmat_d = nc.dram_tensor("cmat", [N, N], BF16, kind="ExternalInput")
    bqk_d = nc.dram_tensor("bqk", [2 * HDIM], F32, kind="ExternalInput")
    y_d = nc.dram_tensor("y", [N, DIN], BF16, kind="ExternalOutput")

    with tile.TileContext(nc) as tc, ExitStack() as top:
        const = top.enter_context(tc.tile_pool(name="const", bufs=1))
        ones_bf = const.tile([128, 1], BF16, tag="ones_bf")
        nc.vector.memset(ones_bf[:], 1.0)
        scr = const.tile([128, 512], BF16, tag="scr")
        nc.vector.memset(scr[:], 0.0)
        bqk_t = const.tile([128, 4], F32, tag="bqk")

        big = top.enter_context(tc.tile_pool(name="big", bufs=1))
        wq = [big.tile([128, 3 * HDIM], BF16, tag=f"wq{c}", name=f"wq{c}")
              for c in range(4)]
        xt = [big.tile([128, N], BF16, tag=f"xt{c}", name=f"xt{c}")
              for c in range(4)]
        wp = [big.tile([128, DIN], BF16, tag=f"wp{c}", name=f"wp{c}")
              for c in range(2)]
        # spread big loads across DMA queues so they land in parallel
        qeng = [nc.gpsimd, nc.sync, nc.scalar]
        for c in range(4):
            qeng[c % 3].dma_start(wq[c][:], wq_d.ap()[c * 128:(c + 1) * 128, :])
            qeng[(c + 1) % 3].dma_start(xt[c][:],
                                        xt_d.ap()[c * 128:(c + 1) * 128, :])
        for j in range(4):
            nc.sync.dma_start(bqk_t[:, j:j + 1],
                              bqk_d.ap()[j * 128:(j + 1) * 128])
        for c in range(2):
            qeng[c].dma_start(wp[c][:], wp_d.ap()[c * 128:(c + 1) * 128, :])

        per = top.enter_context(tc.tile_pool(name="per", bufs=1))
        qkT = [per.tile([128, N], BF16, tag=f"qkT{t}", name=f"qkT{t}")
               for t in range(4)]
        VW = 128  # per-head stationary width: [ones | 63 pad | 64 v dims]
        vv = [per.tile([128, HPC * VW], BF16, tag=f"vv{m}", name=f"vv{m}")
              for m in range(8)]
        Ct = [per.tile([128, N], BF16, tag=f"C{m}", name=f"C{m}")
              for m in range(8)]
        ymT = [per.tile([128, N], BF16, tag=f"ymT{t}", name=f"ymT{t}")
               for t in range(2)]
        for m in range(8):
            qeng[m % 3].dma_start(Ct[m][:], cmat_d.ap()[m * 128:(m + 1) * 128, :])

        with tc.tile_pool(name="psMM", bufs=2, space="PSUM") as psMM, \
             tc.tile_pool(name="psZ", bufs=2, space="PSUM") as psZ, \
             tc.tile_pool(name="psY", bufs=2, space="PSUM") as psY, \
             tc.tile_pool(name="smE1", bufs=16) as smE1, \
             tc.tile_pool(name="smT1", bufs=4) as smT1, \
             tc.tile_pool(name="smT2", bufs=2) as smT2, \
             tc.tile_pool(name="smE2", bufs=3) as smE2, \
             tc.tile_pool(name="smZ", bufs=2) as smZ, \
             tc.tile_pool(name="smO", bufs=4) as smO:

            # HAM pre-warm: ~6us of back-to-back dummy matmuls that only
            # depend on memset data, overlapping the input DMA wait.
            wps = psMM.tile([128, N], F32, tag="ps", name="ps")
            for _ in range(20):
                nc.tensor.matmul(wps[0:1, 0:512], ones_bf[:], scr[:],
                                 start=True, stop=True)

            # ---------- emission helpers ----------
            def emit_qk_tile(t):
                ps = psMM.tile([128, N], F32, tag="ps", name="ps")
                for half in range(2):
                    for c in range(4):
                        nc.tensor.matmul(
                            ps[:, half * 512:(half + 1) * 512],
                            wq[c][:, t * 128:(t + 1) * 128],
                            xt[c][:, half * 512:(half + 1) * 512],
                            start=(c == 0), stop=(c == 3))
                nc.scalar.activation(qkT[t][:], ps[:], AT.Identity,
                                     bias=bqk_t[:, t:t + 1])

            def emit_v_pair(g):  # 2 token-tiles of v per psMM slot
                ps = psMM.tile([128, N], F32, tag="ps", name="ps")
                for i in range(2):
                    m = 2 * g + i
                    pv = ps[:, i * HDIM:(i + 1) * HDIM]
                    for c in range(4):
                        nc.tensor.matmul(pv, xt[c][:, m * 128:(m + 1) * 128],
                                         wq[c][:, 2 * HDIM:3 * HDIM],
                                         start=(c == 0), stop=(c == 3))
                for i in range(2):
                    m = 2 * g + i
                    v3 = vv[m][:].rearrange("p (h e) -> p h e", e=VW)
                    nc.vector.tensor_copy(
                        v3[:, :, 64:VW],
                        ps[:, i * HDIM:(i + 1) * HDIM].rearrange(
                            "p (h e) -> p h e", e=HD))
                    nc.gpsimd.memset(v3[:, :, 0:1], 1.0)
                    nc.gpsimd.memset(v3[:, :, 1:64], 0.0)

            state = {}

            def emit_scores(h, m):
                hp, ho = h // 2, (h % 2) * 64
                st = state[h]
                ps = psMM.tile([128, N], F32, tag="ps", name="ps")
                e1 = smE1.tile([128, N], BF16, tag="e1", name="e1")
                for qh in range(2):
                    nc.tensor.matmul(
                        ps[:, qh * 512:(qh + 1) * 512],
                        qkT[2 + hp][ho:ho + 64, m * 128:(m + 1) * 128],
                        qkT[hp][ho:ho + 64, qh * 512:(qh + 1) * 512],
                        start=True, stop=True)
                nc.scalar.activation(e1[:], ps[:], AT.Exp, scale=SCALE)
                st["e1"][m] = e1

            def emit_sum(h, m):
                st = state[h]
                e1 = st["e1"][m]
                for qh in range(2):
                    nc.tensor.matmul(st["s_ps"][qh][:], ones_bf[:],
                                     e1[:, qh * 512:(qh + 1) * 512],
                                     start=(m == 0), stop=(m == 7))

            def emit_rsb(h):
                st = state[h]
                rs = smZ.tile([1, N], F32, tag="rs", name="rs")
                for qh in range(2):
                    nc.vector.reciprocal_approx_fast(
                        rs[:, qh * 512:(qh + 1) * 512], st["s_ps"][qh][:])
                rs16 = smZ.tile([1, N], BF16, tag="rs16", name="rs16")
                nc.vector.tensor_copy(rs16[:], rs[:])
                rsb = smZ.tile([128, N], BF16, tag="rsb", name="rsb")
                nc.gpsimd.partition_broadcast(rsb[:], rs16[:], channels=128)
                st["rsb"] = rsb
                st["ypv"] = [psY.tile([VW, 512], F32, tag="ypv",
                                      name="ypv") for _ in range(2)]

            def emit_d2_step(h, m):
                st = state[h]
                t1 = smT1.tile([128, N], BF16, tag="t1", name="t1")
                nc.vector.tensor_mul(t1[:], st["e1"][m][:], st["rsb"][:])
                if m % 2 == 0:
                    st["t2p"] = smT2.tile([128, 2 * N], BF16, tag="t2p",
                                          name="t2p")
                sub = m % 2
                t2s = st["t2p"][:, sub * N:(sub + 1) * N]
                nc.vector.tensor_mul(t2s, t1[:], Ct[m][:])
                if sub == 1:
                    e2 = smE2.tile([128, 2 * N], BF16, tag="e2", name="e2")
                    nc.scalar.activation(e2[:], st["t2p"][:], AT.Exp)
                    for s2 in range(2):
                        mm = m - 1 + s2
                        for qh in range(2):
                            nc.tensor.matmul(
                                st["ypv"][qh][:],
                                vv[mm][:, h * VW:(h + 1) * VW],
                                e2[:, s2 * N + qh * 512:s2 * N + (qh + 1) * 512],
                                start=(mm == 0), stop=(mm == 7))

            def emit_ztail(h):
                hp, ho = h // 2, (h % 2) * 64
                st = state.pop(h)
                ypv = st["ypv"]
                rz = smZ.tile([1, N], F32, tag="rz", name="rz")
                for qh in range(2):
                    nc.vector.reciprocal_approx_fast(
                        rz[:, qh * 512:(qh + 1) * 512], ypv[qh][0:1, :])
                zb = smZ.tile([64, N], F32, tag="zb", name="zb")
                nc.gpsimd.partition_broadcast(zb[:], rz[:], channels=64)
                for qh in range(2):
                    nc.vector.tensor_mul(
                        ymT[hp][ho:ho + 64, qh * 512:(qh + 1) * 512],
                        ypv[qh][64:VW, :], zb[:, qh * 512:(qh + 1) * 512])

            # ---------- schedule ----------
            emit_qk_tile(0)
            emit_qk_tile(2)

            filler = ([lambda t=1: emit_qk_tile(t),
                       lambda t=3: emit_qk_tile(t)]
                      + [lambda g=g: emit_v_pair(g) for g in range(4)])

            for h in range(HPC):
                state[h] = {"e1": {}, "s_ps": [
                    psZ.tile([1, 512], F32, tag="s_ps", name="s_ps")
                    for _ in range(2)]}
                for m in range(8):
                    emit_scores(h, m)
                    if m >= 1:
                        emit_sum(h, m - 1)
                    if filler:
                        filler.pop(0)()
                    if h >= 1 and m >= 2:
                        emit_d2_step(h - 1, m - 2)
                emit_sum(h, 7)
                if h >= 1:
                    emit_d2_step(h - 1, 6)
                    emit_d2_step(h - 1, 7)
                    emit_ztail(h - 1)
                emit_rsb(h)
            # ---------- E: partial output projection (pipelined with D2 h3)
            e_ps = {}

            def emit_e_chunk0(np_):
                ps = psMM.tile([128, N], F32, tag="ps", name="ps")
                for i in range(2):
                    nb = 2 * np_ + i
                    nc.tensor.matmul(ps[:, i * 512:(i + 1) * 512],
                                     ymT[0][:, nb * 128:(nb + 1) * 128],
                                     wp[0][:], start=True, stop=False)
                e_ps[np_] = ps

            def emit_e_finish(np_):
                if np_ not in e_ps:
                    emit_e_chunk0(np_)
                ps = e_ps[np_]
                for i in range(2):
                    nb = 2 * np_ + i
                    nc.tensor.matmul(ps[:, i * 512:(i + 1) * 512],
                                     ymT[1][:, nb * 128:(nb + 1) * 128],
                                     wp[1][:], start=False, stop=True)
                yo = smO.tile([128, N], BF16, tag="yo", name="yo")
                if np_ % 2 == 0:
                    nc.scalar.copy(yo[:], ps[:])
                else:
                    nc.vector.tensor_copy(yo[:], ps[:])
                for i in range(2):
                    nb = 2 * np_ + i
                    qeng[nb % 3].dma_start(y_d.ap()[nb * 128:(nb + 1) * 128, :],
                                           yo[:, i * 512:(i + 1) * 512])

            emit_e_chunk0(0)
            emit_e_chunk0(1)
            for m in range(8):
                emit_d2_step(HPC - 1, m)
            emit_ztail(HPC - 1)
            for np_ in range(4):
                emit_e_finish(np_)

    nc.compile()
    return nc


def _host_fold(code, w_c, W_qkv, b_qkv, W_proj, b_proj,
               ln_qkv_g, ln_qkv_b, ln_proj_g, ln_proj_b):
    """Compute modulation vectors on host and fold into weights/biases."""
    cm0 = (w_c @ code).T  # [NF, DIN]
    def ln(v, g, b, eps=1e-5):
        mu = v.mean(-1, keepdims=True)
        var = v.var(-1, keepdims=True)
        return (v - mu) / np.sqrt(var + eps) * g + b
    cmq = ln(cm0, ln_qkv_g, ln_qkv_b)    # [NF, DIN]
    cmp_ = ln(cm0, ln_proj_g, ln_proj_b)
    bv = b_qkv[2 * DIN:3 * DIN]
    bias_f = b_proj[None, :] + (bv[None, :] * cmp_) @ W_proj.T  # [NF, DIN]
    return cmq, cmp_, bias_f


def make_in_maps(x, compatibility, code, w_c, W_qkv, b_qkv, W_proj, b_proj,
                 ln_qkv_g, ln_qkv_b, ln_proj_g, ln_proj_b):
    import ml_dtypes
    bf = ml_dtypes.bfloat16
    x = np.asarray(x, np.float32)
    compatibility = np.asarray(compatibility, np.float32)
    code = np.asarray(code, np.float32)
    w_c = np.asarray(w_c, np.float32)
    W_qkv = np.asarray(W_qkv, np.float32)
    b_qkv = np.asarray(b_qkv, np.float32)
    W_proj = np.asarray(W_proj, np.float32)
    b_proj = np.asarray(b_proj, np.float32)

    cmq, cmp_, bias_f = _host_fold(
        code, w_c, W_qkv, b_qkv, W_proj, b_proj,
        np.asarray(ln_qkv_g, np.float32), np.asarray(ln_qkv_b, np.float32),
        np.asarray(ln_proj_g, np.float32), np.asarray(ln_proj_b, np.float32))

    xT = np.ascontiguousarray(x[0].T.astype(bf))      # [512, 1024] bf16
    compb = compatibility[0].astype(bf).astype(np.float32)
    cmat = np.ascontiguousarray((compb.T @ compb).astype(bf))  # [1024,1024]
    in_maps = []
    for core in range(N_CORES):
        f, hh = core // 2, core % 2
        Wq_f = W_qkv * cmq[f][None, :]                # [1536, 512]
        rows = np.r_[hh * HDIM:(hh + 1) * HDIM,
                     DIN + hh * HDIM:DIN + (hh + 1) * HDIM,
                     2 * DIN + hh * HDIM:2 * DIN + (hh + 1) * HDIM]
        wq = np.ascontiguousarray(Wq_f[rows].T.astype(bf))   # [512, 768]
        Wp_f = W_proj * cmp_[f][None, :]              # [512, 512]
        wp = np.ascontiguousarray(
            Wp_f[:, hh * HDIM:(hh + 1) * HDIM].T.astype(bf))  # [256, 512]
        bqk = np.ascontiguousarray(
            b_qkv[np.r_[hh * HDIM:(hh + 1) * HDIM,
                        DIN + hh * HDIM:DIN + (hh + 1) * HDIM]])  # [512]
        in_maps.append(dict(xt=xT, wq=wq, wp=wp, cmat=cmat, bqk=bqk))
    _CACHE["bias_f"] = bias_f
    return in_maps


def kernel(**inputs) -> np.ndarray:
    from concourse.bass_utils import run_bass_kernel_spmd
    if "nc" not in _CACHE:
        _CACHE["nc"] = build_nc()
    nc = _CACHE["nc"]
    in_maps = make_in_maps(**inputs)
    bias_f = _CACHE["bias_f"]
    res = run_bass_kernel_spmd(nc, in_maps, core_ids=list(range(N_CORES)))
    out = np.empty((1, NF, N, DIN), np.float32)
    for f in range(NF):
        p0 = np.asarray(res.results[2 * f]["y"]).astype(np.float32)
        p1 = np.asarray(res.results[2 * f + 1]["y"]).astype(np.float32)
        out[0, f] = p0 + p1 + bias_f[f][None, :]
    return out
